# revision 21
# baseline (speedup 1.0000x reference)
"""Cross-attention Trainium2 kernel (8 NeuronCores, Bass/Tile).

Problem (hardcoded): B=2, SQ=SKV=2048, D=1024, H=16 heads, HD=64.
  q  = query @ Wq + bq
  kv = context @ Wkv + bkv ; split into k, v per head
  o  = softmax(q k^T / sqrt(hd) + mask) v         (mask: -inf where True)
  out = o @ Wout + bout

Sharding: core c = (b, g) with b = c // 4 (batch), g = c % 4 (head group of 4).
Each core computes its batch's attention for its 4 heads and the partial out
projection (Wout rows for those heads); host sums the 4 partials per batch and
adds bout (linearity of the out projection).

Everything on-chip runs "transposed" (feature dim on partitions, tokens on the
free dim), so the host passes query/context transposed and gets the partial
output transposed back. Softmax uses no max subtraction (scores are ~N(0,1)
here; exp is safe in fp32) and folds masking into V: v rows are scaled by
keep=1-mask and an extra "keep" column of V yields the softmax denominator via
the same PE accumulation.

All matmuls run in bfloat16 (same PE rate as fp32r but half the DMA/SBUF
traffic and FWL weight loads); PSUM accumulation stays fp32.

The scores matmul contracts over only the 64-wide head dim, so it uses half
the PE array. The two heads of a pair sit at PE row bands 0-63 / 64-127 (the
kt/qt layout puts head 2p at partitions 0-63 and head 2p+1 at 64-127), and
their score matmuls are interleaved instruction-by-instruction: the hardware
runs the two bands concurrently (PE row tiling), nearly halving score time.
Attention runs on 256-wide q chunks so both heads' AV accumulators fit in a
single PSUM bank, keeping the whole working set inside the 8 banks.
"""

import sys

sys.path.insert(0, "/opt/trn_rl_repo")

import numpy as np

B, SQ, SKV, D, H, HD = 2, 2048, 2048, 1024, 16, 64
HG = 4                # heads per core
COLS = HG * HD        # 256 projected columns per core (per q/k/v)
DK = D // 128         # 8 contraction tiles
SQC = 256             # sq chunk (half psum bank at fp32)
NSQC = SQ // SQC
SKC = 512             # skv chunk for kv projection
NSKC = SKV // SKC
NJ = SKV // 128       # 16 skv tiles for attention

_CACHE = {}


def _build(with_bias=False):
    import concourse.bacc as bacc
    import concourse.mybir as mybir
    import concourse.tile as tile

    F32 = mybir.dt.float32
    BF16 = mybir.dt.bfloat16
    EXP = mybir.ActivationFunctionType.Exp

    nc = bacc.Bacc()

    # ---- DRAM I/O (per core) ----
    qryT = nc.dram_tensor("qryT", [D, SQ], BF16, kind="ExternalInput")
    ctxT = nc.dram_tensor("ctxT", [D, SKV], BF16, kind="ExternalInput")
    wq = nc.dram_tensor("wq", [D, COLS], BF16, kind="ExternalInput")
    wk = nc.dram_tensor("wk", [D, COLS], BF16, kind="ExternalInput")
    wv = nc.dram_tensor("wv", [D, COLS], BF16, kind="ExternalInput")
    wout = nc.dram_tensor("wout", [COLS, D], BF16, kind="ExternalInput")
    bq = nc.dram_tensor("bq", [1, COLS], BF16, kind="ExternalInput")
    bk = nc.dram_tensor("bk", [1, COLS], BF16, kind="ExternalInput")
    bv = nc.dram_tensor("bv", [1, COLS], BF16, kind="ExternalInput")
    ones = nc.dram_tensor("ones", [1, SKC], BF16, kind="ExternalInput")
    keep = nc.dram_tensor("keep", [128, NJ], F32, kind="ExternalInput")
    outT = nc.dram_tensor("outT", [D, SQ], BF16, kind="ExternalOutput")

    with tile.TileContext(nc) as tc:
        with (
            tc.tile_pool(name="w", bufs=1) as wp,
            tc.tile_pool(name="big", bufs=1) as bigp,
            tc.tile_pool(name="strips", bufs=3) as sp,
            tc.tile_pool(name="work", bufs=1) as workp,
            tc.tile_pool(name="ps", bufs=1, space="PSUM") as psp,
        ):
            # ---- weights / constants ----
            wq_sb = wp.tile([128, DK, COLS], BF16)
            wk_sb = wp.tile([128, DK, COLS], BF16)
            wv_sb = wp.tile([128, DK, COLS], BF16)
            wout_sb = wp.tile([128, 2, D], BF16)
            bq_sb = wp.tile([1, COLS], BF16)
            bk_sb = wp.tile([1, COLS], BF16)
            bv_sb = wp.tile([1, COLS], BF16)
            ones_sb = wp.tile([1, SKC], BF16)
            keep_sb = wp.tile([128, NJ], F32)

            ctxT_r = ctxT.ap().rearrange("(t p) s -> p t s", p=128)
            qryT_r = qryT.ap().rearrange("(t p) s -> p t s", p=128)
            outT_r = outT.ap().rearrange("(t p) s -> p t s", p=128)

            # Startup-critical DMAs split per d-tile so the first kT matmul
            # (needs wk d=0 + ctx d=0 only) starts after ~0.2 MB.
            wk_r = wk.ap().rearrange("(t p) m -> p t m", p=128)
            ctx0_sb = sp.tile([128, DK, SKC], BF16, tag="strip")
            nc.sync.dma_start(wk_sb[:, 0:1, :], wk_r[:, 0:1, :])
            nc.sync.dma_start(ctx0_sb[:, 0:1, :], ctxT_r[:, 0:1, 0:SKC])
            nc.sync.dma_start(bk_sb[:], bk.ap())
            nc.sync.dma_start(ones_sb[:], ones.ap())
            for d in range(1, DK):
                nc.sync.dma_start(wk_sb[:, d:d + 1, :], wk_r[:, d:d + 1, :])
                nc.sync.dma_start(ctx0_sb[:, d:d + 1, :], ctxT_r[:, d:d + 1, 0:SKC])
            # qproj(0) runs between kT-jc0 and the first scores, so its
            # inputs (qry0, wq) come right after the kT inputs.
            wq_r = wq.ap().rearrange("(t p) m -> p t m", p=128)
            qry0_sb = sp.tile([128, DK, SQC], BF16, tag="strip", name="qry0_sb")
            nc.sync.dma_start(bq_sb[:], bq.ap())
            for d in range(DK):
                nc.sync.dma_start(wq_sb[:, d:d + 1, :], wq_r[:, d:d + 1, :])
                nc.sync.dma_start(qry0_sb[:, d:d + 1, :], qryT_r[:, d:d + 1, 0:SQC])
            nc.sync.dma_start(wv_sb[:], wv.ap().rearrange("(t p) m -> p t m", p=128))
            nc.sync.dma_start(bv_sb[:], bv.ap())
            nc.sync.dma_start(keep_sb[:], keep.ap())
            # pre-issue the remaining ctx strips so they queue ahead of wout
            # and the later qry strips
            strip_tiles = [ctx0_sb]
            for jc in range(1, NSKC):
                st = sp.tile([128, DK, SKC], BF16, tag="strip", name=f"ctx{jc}_sb")
                nc.sync.dma_start(st[:], ctxT_r[:, :, jc * SKC:(jc + 1) * SKC])
                strip_tiles.append(st)

            # ---- persistent activations ----
            kt_sb = bigp.tile([128, 2, SKV], BF16)        # k^T, head pair per 64-row band
            v_sb = bigp.tile([128, NJ, HG, HD + 1], BF16)  # v + keep column, [skv%128, j, h, :]
            qt_all = bigp.tile([128, 2, SQ], BF16)         # q^T for all chunks

            # ============ Phase K as a generator (interleaved into qc0) ============
            def emit_K_kT(jc):
                ctx_sb = strip_tiles[jc]
                pk = psp.tile([128, 2, SKC], F32, tag="mm", bufs=2, name="pk")
                for cc in range(2):
                    for d in range(DK):
                        nc.tensor.matmul(
                            pk[:, cc, :],
                            wk_sb[:, d, cc * 128:(cc + 1) * 128],
                            ctx_sb[:, d, :],
                            start=(d == 0), stop=(not with_bias and d == DK - 1),
                        )
                    if with_bias:
                        nc.tensor.matmul(
                            pk[:, cc, :],
                            bk_sb[0:1, cc * 128:(cc + 1) * 128],
                            ones_sb[0:1, :],
                            start=False, stop=True,
                        )
                nc.vector.tensor_copy(kt_sb[:, :, jc * SKC:(jc + 1) * SKC], pk[:])

            def emit_K_v(jc):
                ctx_sb = strip_tiles[jc]
                for jjp in range(2):
                    pv = psp.tile([128, 2, SKC], F32, tag="mm", bufs=2, name="pv")
                    for sub in range(2):
                        jj = jjp * 2 + sub
                        for d in range(DK):
                            nc.tensor.matmul(
                                pv[:, sub, 0:COLS],
                                ctx_sb[:, d, jj * 128:(jj + 1) * 128],
                                wv_sb[:, d, :],
                                start=(d == 0), stop=(not with_bias and d == DK - 1),
                            )
                        if with_bias:
                            nc.tensor.matmul(
                                pv[:, sub, 0:COLS],
                                ones_sb[0:1, 0:128],
                                bv_sb[0:1, :],
                                start=False, stop=True,
                            )
                    for sub in range(2):
                        jj = jjp * 2 + sub
                        j = jc * 4 + jj
                        nc.vector.tensor_scalar_mul(
                            v_sb[:, j, :, 0:HD],
                            pv[:, sub, 0:COLS].rearrange("p (h e) -> p h e", h=HG),
                            keep_sb[:, j:j + 1],
                        )
                        for h in range(HG):
                            nc.vector.tensor_copy(
                                v_sb[:, j, h, HD:HD + 1], keep_sb[:, j:j + 1]
                            )

            def gen_phaseK_rest():
                for jc in range(1, NSKC):
                    if jc == NSKC - 1:
                        nc.sync.dma_start(wout_sb[:], wout.ap().rearrange("(t p) m -> p t m", p=128))
                    emit_K_kT(jc)
                    emit_K_v(jc)
                    yield

            # ====== Phase A: software-pipelined attention ======
            # Filler generators keep each PSUM tile's write->read window
            # atomic (no yield between a tile's matmuls and the op that
            # consumes it): a paused generator must never hold a psum slot
            # whose reader is unemitted, or another allocator on the same
            # tag could deadlock the in-order PE queue.
            def gen_qproj(qc, qry_sb=None):
                if qry_sb is None:
                    qry_sb = sp.tile([128, DK, SQC], BF16, tag="strip", name="qry_sb")
                    nc.sync.dma_start(qry_sb[:], qryT_r[:, :, qc * SQC:(qc + 1) * SQC])
                yield
                for cc in range(2):
                    pq = psp.tile([128, SQC], F32, tag="av", bufs=1, name="pq")
                    for d in range(DK):
                        nc.tensor.matmul(
                            pq[:],
                            wq_sb[:, d, cc * 128:(cc + 1) * 128],
                            qry_sb[:, d, :],
                            start=(d == 0), stop=(not with_bias and d == DK - 1),
                        )
                    if with_bias:
                        nc.tensor.matmul(
                            pq[:],
                            bq_sb[0:1, cc * 128:(cc + 1) * 128],
                            ones_sb[0:1, 0:SQC],
                            start=False, stop=True,
                        )
                    nc.vector.tensor_copy(
                        qt_all[:, cc, qc * SQC:(qc + 1) * SQC], pq[:]
                    )
                    yield
                qdone[qc] = True

            def gen_outproj(qc, otn, epilogue=False):
                odone[qc] = False
                for m in range(8):
                    ptag = ("av", "mm")[m % 2] if epilogue else "av"
                    pf = psp.tile([128, SQC], F32, tag=ptag, bufs=1 if ptag == "av" else 2, name="pf")
                    nc.tensor.matmul(
                        pf[:],
                        wout_sb[:, 0, m * 128:(m + 1) * 128],
                        otn[:, 0, :],
                        start=True, stop=False,
                    )
                    nc.tensor.matmul(
                        pf[:],
                        wout_sb[:, 1, m * 128:(m + 1) * 128],
                        otn[:, 1, :],
                        start=False, stop=True,
                    )
                    fin = workp.tile([128, SQC], BF16, tag="fin", bufs=4)
                    if epilogue:
                        nc.scalar.copy(fin[:], pf[:])
                    else:
                        nc.vector.tensor_copy(fin[:], pf[:])
                    nc.sync.dma_start(
                        outT_r[:, m, qc * SQC:(qc + 1) * SQC], fin[:]
                    )
                    yield
                odone[qc] = True

            filler = []
            qdone = {0: True}
            odone = {}

            def emit_filler(budget):
                while budget > 0 and filler:
                    try:
                        next(filler[0])
                        budget -= 1
                    except StopIteration:
                        filler.pop(0)

            emit_K_kT(0)
            for _ in gen_qproj(0, qry0_sb):
                pass
            kgen = gen_phaseK_rest()

            # 16 j-tiles per head in 3 groups; first group smaller so the
            # phase-K interleave (kgen) paces the startup DMAs.
            GROUPS = (4, 6, 6)
            kdone = [1]  # K-jc0 emitted in the prologue
            otn_prev = None
            for qc in range(NSQC):
                if qc + 1 < NSQC:
                    qdone[qc + 1] = False
                    filler.append(gen_qproj(qc + 1))
                if otn_prev is not None:
                    filler.append(gen_outproj(qc - 1, otn_prev))
                # hard guarantees: qproj(qc) must be fully emitted before this
                # chunk's scores reference its qt columns, and outproj(qc-2)
                # before this chunk's otn slot (bufs=2 rotation) is rewritten
                # by the (front-inserted) norm fillers
                while not qdone[qc]:
                    emit_filler(50)
                while not odone.get(qc - 2, True):
                    emit_filler(50)
                qt = qt_all[:, :, qc * SQC:(qc + 1) * SQC]
                otn = workp.tile([128, 2, SQC], BF16, tag="otn", bufs=2)
                for pair in range(2):
                    startup = qc == 0 and pair == 0
                    he, ho = 2 * pair, 2 * pair + 1
                    # both heads' AV accumulators share one PSUM bank;
                    # allocated lazily at the first AV group so the bufs=1
                    # rotation waits on the previous pair's (already-emitted)
                    # ot copy
                    pav_h = [None]

                    def get_pav(pav_h=pav_h):
                        if pav_h[0] is None:
                            pav_h[0] = psp.tile(
                                [HD + 1, 2, SQC], F32, tag="pav", bufs=1, name="pav"
                            )
                        return pav_h[0]

                    def emit_av(prev, he=he, ho=ho):
                        gs0, jbase0, pt_e0, pt_o0 = prev
                        pav = get_pav()
                        for hh, h, pt0 in ((0, he, pt_e0), (1, ho, pt_o0)):
                            for sub in range(gs0):
                                j = jbase0 + sub
                                # start=True clears has_written for the WHOLE
                                # bank, so only the pair's first matmul may set
                                # it; head-odd's j=0 matmul overwrites its
                                # (bit-cleared) region via flags=0 semantics.
                                nc.tensor.matmul(
                                    pav[:, hh, :],
                                    v_sb[:, j, h, :],
                                    pt0[:, sub, :],
                                    start=(j == 0 and hh == 0),
                                    stop=(j == NJ - 1),
                                    skip_group_check=(j == 0 and hh == 1),
                                )

                    prev = None
                    jbase = 0
                    for gi, gs in enumerate(GROUPS):
                        if startup:
                            # emit K-jc sections before the groups needing them
                            need = (jbase + gs - 1) // 4
                            while kdone[0] <= need:
                                next(kgen)
                                kdone[0] += 1
                        ps_e = psp.tile([128, 6, SQC], F32, tag="mm", bufs=2, name="ps_e")
                        ps_o = psp.tile([128, 6, SQC], F32, tag="mm", bufs=2, name="ps_o")
                        # interleave the two heads' score matmuls so the two
                        # 64-row PE bands run concurrently
                        for sub in range(gs):
                            j = jbase + sub
                            nc.tensor.matmul(
                                ps_e[:, sub, :],
                                kt_sb[0:64, pair, j * 128:(j + 1) * 128],
                                qt[0:64, pair, :],
                                start=True, stop=True,
                            )
                            nc.tensor.matmul(
                                ps_o[:, sub, :],
                                kt_sb[64:128, pair, j * 128:(j + 1) * 128],
                                qt[64:128, pair, :],
                                start=True, stop=True,
                            )
                        pt_e = workp.tile([128, 6, SQC], BF16, tag="pt", bufs=4, name="pt_e")
                        pt_o = workp.tile([128, 6, SQC], BF16, tag="pt", bufs=4, name="pt_o")
                        nc.scalar.activation(pt_e[:, 0:gs, :], ps_e[:, 0:gs, :], EXP)
                        if startup and gi == 0:
                            emit_K_v(0)
                        nc.scalar.activation(pt_o[:, 0:gs, :], ps_o[:, 0:gs, :], EXP)
                        if prev is not None:
                            emit_av(prev)
                        prev = (gs, jbase, pt_e, pt_o)
                        jbase += gs
                        if not startup:
                            emit_filler(3)
                    emit_av(prev)
                    if not startup:
                        emit_filler(1)

                    # normalization (divide by the keep-column accumulation)
                    # is deferred: emitted as front-of-queue filler during the
                    # next pair's groups, so the PE never waits on the DVE
                    # ot-copy / reciprocal chain
                    def gen_norm(pair=pair, pav=get_pav(), otn=otn):
                        ot = workp.tile([HD + 1, 2, SQC], F32, tag="ot", bufs=2)
                        nc.vector.tensor_copy(ot[:], pav[:])
                        rcp = workp.tile([1, 2, SQC], BF16, tag="rcp", bufs=2)
                        with nc.allow_low_precision(reason="bf16 reciprocal for softmax denom"):
                            nc.vector.reciprocal(rcp[:], ot[HD:HD + 1, :, :])
                        yield
                        # one matmul broadcasts both heads' reciprocals, and
                        # both muls (its readers) are emitted atomically with it
                        pbc = psp.tile([HD, 2, SQC], F32, tag="av", bufs=1, name="pbc")
                        nc.tensor.matmul(
                            pbc[:, :, :], ones_sb[0:1, 0:HD], rcp[0:1, :, :],
                            start=True, stop=True,
                        )
                        for hh in range(2):
                            po = hh * 64
                            nc.vector.tensor_mul(
                                otn[po:po + 64, pair, :], ot[0:HD, hh, :], pbc[:, hh, :]
                            )
                        yield

                    filler.insert(0, gen_norm())
                    if startup:
                        # keep the startup pair's norm from lagging behind the
                        # whole qproj(1) generator
                        emit_filler(2)
                otn_prev = otn

            # drain remaining filler, then the final chunk's out-projection
            emit_filler(10 ** 9)
            for _ in gen_outproj(NSQC - 1, otn_prev, epilogue=True):
                pass

    nc.compile()
    return nc


def _get_nc(with_bias=False):
    key = f"nc{int(with_bias)}"
    if key not in _CACHE:
        _CACHE[key] = _build(with_bias)
    return _CACHE[key]


LAST_RESULTS = None
LAST_IN_MAPS = None


def kernel(query, context, mask, Wq, bq, Wkv, bkv, Wout, bout, num_heads):
    import os
    import ml_dtypes
    from concourse.bass_utils import run_bass_kernel_spmd

    BF = ml_dtypes.bfloat16

    query = np.asarray(query, dtype=np.float32)
    context = np.asarray(context, dtype=np.float32)
    mask = np.asarray(mask)
    Wq = np.asarray(Wq, dtype=np.float32)
    bq_v = np.asarray(bq, dtype=np.float32)
    Wkv = np.asarray(Wkv, dtype=np.float32)
    bkv_v = np.asarray(bkv, dtype=np.float32)
    Wout = np.asarray(Wout, dtype=np.float32)
    bout_v = np.asarray(bout, dtype=np.float32)
    assert int(num_heads) == H

    scale = np.float32(HD ** -0.5)
    Wq_s = Wq * scale
    bq_s = bq_v * scale
    Wk = Wkv[:, :D]
    Wv = Wkv[:, D:]
    bk_v = bkv_v[:D]
    bv_v = bkv_v[D:]
    keep_f = 1.0 - mask.astype(np.float32)          # [B, SKV]
    ones_r = np.ones((1, SKC), dtype=BF)

    def bf(x):
        return np.ascontiguousarray(x.astype(BF))

    with_bias = bool(np.any(bq_s) or np.any(bk_v) or np.any(bv_v))
    nc = _get_nc(with_bias)
    in_maps = []
    for c in range(8):
        b, g = c // 4, c % 4
        cs = slice(g * COLS, (g + 1) * COLS)
        in_maps.append({
            "qryT": bf(query[b].T),
            "ctxT": bf(context[b].T),
            "wq": bf(Wq_s[:, cs]),
            "wk": bf(Wk[:, cs]),
            "wv": bf(Wv[:, cs]),
            "wout": bf(Wout[cs, :]),
            "bq": bf(bq_s[cs][None, :]),
            "bk": bf(bk_v[cs][None, :]),
            "bv": bf(bv_v[cs][None, :]),
            "ones": ones_r,
            "keep": np.ascontiguousarray(keep_f[b].reshape(NJ, 128).T),
        })

    trace = bool(int(os.environ.get("KERNEL_TRACE", "0")))
    res = run_bass_kernel_spmd(nc, in_maps, core_ids=list(range(8)), trace=trace)
    global LAST_RESULTS, LAST_IN_MAPS
    LAST_RESULTS = res
    LAST_IN_MAPS = in_maps

    out = np.empty((B, SQ, D), dtype=np.float32)
    for b in range(B):
        acc = np.zeros((D, SQ), dtype=np.float32)
        for g in range(4):
            acc += res.results[b * 4 + g]["outT"].astype(np.float32)
        out[b] = acc.T + bout_v[None, :]
    return out


# revision 24
# speedup vs baseline: 1.1583x; 1.1583x over previous
"""Cross-attention Trainium2 kernel (8 NeuronCores, Bass/Tile).

Problem (hardcoded): B=2, SQ=SKV=2048, D=1024, H=16 heads, HD=64.
  q  = query @ Wq + bq
  kv = context @ Wkv + bkv ; split into k, v per head
  o  = softmax(q k^T / sqrt(hd) + mask) v         (mask: -inf where True)
  out = o @ Wout + bout

Sharding: core c = (b, g) with b = c // 4 (batch), g = c % 4 (head group of 4).
Each core computes its batch's attention for its 4 heads and the partial out
projection (Wout rows for those heads); host sums the 4 partials per batch and
adds bout (linearity of the out projection).

Everything on-chip runs "transposed" (feature dim on partitions, tokens on the
free dim), so the host passes query/context transposed and gets the partial
output transposed back. Softmax uses no max subtraction (scores are ~N(0,1)
here; exp is safe in fp32) and folds masking into V: v rows are scaled by
keep=1-mask and an extra "keep" column of V yields the softmax denominator via
the same PE accumulation.

All matmuls run in bfloat16 (same PE rate as fp32r but half the DMA/SBUF
traffic and FWL weight loads); PSUM accumulation stays fp32.

The scores matmul contracts over only the 64-wide head dim, so it uses half
the PE array. The two heads of a pair sit at PE row bands 0-63 / 64-127 (the
kt/qt layout puts head 2p at partitions 0-63 and head 2p+1 at 64-127), and
their score matmuls are interleaved instruction-by-instruction: the hardware
runs the two bands concurrently (PE row tiling), nearly halving score time.
Attention runs on 256-wide q chunks so both heads' AV accumulators fit in a
single PSUM bank, keeping the whole working set inside the 8 banks.
"""

import sys

sys.path.insert(0, "/opt/trn_rl_repo")

import numpy as np

B, SQ, SKV, D, H, HD = 2, 2048, 2048, 1024, 16, 64
HG = 4                # heads per core
COLS = HG * HD        # 256 projected columns per core (per q/k/v)
DK = D // 128         # 8 contraction tiles
SQC = 256             # sq chunk (half psum bank at fp32)
NSQC = SQ // SQC
SKC = 512             # skv chunk for kv projection
NSKC = SKV // SKC
NJ = SKV // 128       # 16 skv tiles for attention

_CACHE = {}


def _build(with_bias=False):
    import concourse.bacc as bacc
    import concourse.mybir as mybir
    import concourse.tile as tile

    F32 = mybir.dt.float32
    BF16 = mybir.dt.bfloat16
    EXP = mybir.ActivationFunctionType.Exp

    nc = bacc.Bacc()

    # ---- DRAM I/O (per core) ----
    qryT = nc.dram_tensor("qryT", [D, SQ], BF16, kind="ExternalInput")
    ctxT = nc.dram_tensor("ctxT", [D, SKV], BF16, kind="ExternalInput")
    wq = nc.dram_tensor("wq", [D, COLS], BF16, kind="ExternalInput")
    wk = nc.dram_tensor("wk", [D, COLS], BF16, kind="ExternalInput")
    wv = nc.dram_tensor("wv", [D, COLS], BF16, kind="ExternalInput")
    wout = nc.dram_tensor("wout", [COLS, D], BF16, kind="ExternalInput")
    bq = nc.dram_tensor("bq", [1, COLS], BF16, kind="ExternalInput")
    bk = nc.dram_tensor("bk", [1, COLS], BF16, kind="ExternalInput")
    bv = nc.dram_tensor("bv", [1, COLS], BF16, kind="ExternalInput")
    ones = nc.dram_tensor("ones", [1, SKC], BF16, kind="ExternalInput")
    keep = nc.dram_tensor("keep", [128, NJ], F32, kind="ExternalInput")
    outT = nc.dram_tensor("outT", [D, SQ], BF16, kind="ExternalOutput")

    with tile.TileContext(nc) as tc:
        with (
            tc.tile_pool(name="w", bufs=1) as wp,
            tc.tile_pool(name="big", bufs=1) as bigp,
            tc.tile_pool(name="strips", bufs=3) as sp,
            tc.tile_pool(name="work", bufs=1) as workp,
            tc.tile_pool(name="ps", bufs=1, space="PSUM") as psp,
        ):
            # ---- weights / constants ----
            wq_sb = wp.tile([128, DK, COLS], BF16)
            wk_sb = wp.tile([128, DK, COLS], BF16)
            wv_sb = wp.tile([128, DK, COLS], BF16)
            wout_sb = wp.tile([128, 2, D], BF16)
            bq_sb = wp.tile([1, COLS], BF16)
            bk_sb = wp.tile([1, COLS], BF16)
            bv_sb = wp.tile([1, COLS], BF16)
            ones_sb = wp.tile([1, SKC], BF16)
            keep_sb = wp.tile([128, NJ], F32)

            ctxT_r = ctxT.ap().rearrange("(t p) s -> p t s", p=128)
            qryT_r = qryT.ap().rearrange("(t p) s -> p t s", p=128)
            outT_r = outT.ap().rearrange("(t p) s -> p t s", p=128)

            # Startup-critical DMAs split per d-tile so the first kT matmul
            # (needs wk d=0 + ctx d=0 only) starts after ~0.2 MB.
            wk_r = wk.ap().rearrange("(t p) m -> p t m", p=128)
            ctx0_sb = sp.tile([128, DK, SKC], BF16, tag="strip")
            nc.sync.dma_start(wk_sb[:, 0:1, :], wk_r[:, 0:1, :])
            nc.sync.dma_start(ctx0_sb[:, 0:1, :], ctxT_r[:, 0:1, 0:SKC])
            nc.sync.dma_start(bk_sb[:], bk.ap())
            nc.sync.dma_start(ones_sb[:], ones.ap())
            for d in range(1, DK):
                nc.sync.dma_start(wk_sb[:, d:d + 1, :], wk_r[:, d:d + 1, :])
                nc.sync.dma_start(ctx0_sb[:, d:d + 1, :], ctxT_r[:, d:d + 1, 0:SKC])
            # qproj(0) runs between kT-jc0 and the first scores, so its
            # inputs (qry0, wq) come right after the kT inputs.
            wq_r = wq.ap().rearrange("(t p) m -> p t m", p=128)
            qry0_sb = sp.tile([128, DK, SQC], BF16, tag="strip", name="qry0_sb")
            nc.sync.dma_start(bq_sb[:], bq.ap())
            for d in range(DK):
                nc.sync.dma_start(wq_sb[:, d:d + 1, :], wq_r[:, d:d + 1, :])
                nc.sync.dma_start(qry0_sb[:, d:d + 1, :], qryT_r[:, d:d + 1, 0:SQC])
            nc.sync.dma_start(wv_sb[:], wv.ap().rearrange("(t p) m -> p t m", p=128))
            nc.sync.dma_start(bv_sb[:], bv.ap())
            nc.sync.dma_start(keep_sb[:], keep.ap())
            # pre-issue the remaining ctx strips so they queue ahead of wout
            # and the later qry strips
            strip_tiles = [ctx0_sb]
            for jc in range(1, NSKC):
                st = sp.tile([128, DK, SKC], BF16, tag="strip", name=f"ctx{jc}_sb")
                nc.sync.dma_start(st[:], ctxT_r[:, :, jc * SKC:(jc + 1) * SKC])
                strip_tiles.append(st)

            # ---- persistent activations ----
            kt_sb = bigp.tile([128, 2, SKV], BF16)        # k^T, head pair per 64-row band
            v_sb = bigp.tile([128, NJ, HG, HD + 1], BF16)  # v + keep column, [skv%128, j, h, :]
            qt_all = bigp.tile([128, 2, SQ], BF16)         # q^T for all chunks

            # ============ Phase K as a generator (interleaved into qc0) ============
            def emit_K_kT(jc):
                ctx_sb = strip_tiles[jc]
                pk = psp.tile([128, 2, SKC], F32, tag="mm", bufs=2, name="pk")
                for cc in range(2):
                    for d in range(DK):
                        nc.tensor.matmul(
                            pk[:, cc, :],
                            wk_sb[:, d, cc * 128:(cc + 1) * 128],
                            ctx_sb[:, d, :],
                            start=(d == 0), stop=(not with_bias and d == DK - 1),
                        )
                    if with_bias:
                        nc.tensor.matmul(
                            pk[:, cc, :],
                            bk_sb[0:1, cc * 128:(cc + 1) * 128],
                            ones_sb[0:1, :],
                            start=False, stop=True,
                        )
                nc.vector.tensor_copy(kt_sb[:, :, jc * SKC:(jc + 1) * SKC], pk[:])

            def emit_K_v(jc):
                ctx_sb = strip_tiles[jc]
                for jjp in range(2):
                    pv = psp.tile([128, 2, SKC], F32, tag="mm", bufs=2, name="pv")
                    for sub in range(2):
                        jj = jjp * 2 + sub
                        for d in range(DK):
                            nc.tensor.matmul(
                                pv[:, sub, 0:COLS],
                                ctx_sb[:, d, jj * 128:(jj + 1) * 128],
                                wv_sb[:, d, :],
                                start=(d == 0), stop=(not with_bias and d == DK - 1),
                            )
                        if with_bias:
                            nc.tensor.matmul(
                                pv[:, sub, 0:COLS],
                                ones_sb[0:1, 0:128],
                                bv_sb[0:1, :],
                                start=False, stop=True,
                            )
                    for sub in range(2):
                        jj = jjp * 2 + sub
                        j = jc * 4 + jj
                        nc.vector.tensor_scalar_mul(
                            v_sb[:, j, :, 0:HD],
                            pv[:, sub, 0:COLS].rearrange("p (h e) -> p h e", h=HG),
                            keep_sb[:, j:j + 1],
                        )
                        for h in range(HG):
                            nc.vector.tensor_copy(
                                v_sb[:, j, h, HD:HD + 1], keep_sb[:, j:j + 1]
                            )

            def gen_phaseK_rest():
                for jc in range(1, NSKC):
                    if jc == NSKC - 1:
                        nc.sync.dma_start(wout_sb[:], wout.ap().rearrange("(t p) m -> p t m", p=128))
                    emit_K_kT(jc)
                    emit_K_v(jc)
                    yield

            # ====== Phase A: software-pipelined attention ======
            # Filler generators keep each PSUM tile's write->read window
            # atomic (no yield between a tile's matmuls and the op that
            # consumes it): a paused generator must never hold a psum slot
            # whose reader is unemitted, or another allocator on the same
            # tag could deadlock the in-order PE queue.
            def gen_qproj(qc, qry_sb=None):
                if qry_sb is None:
                    qry_sb = sp.tile([128, DK, SQC], BF16, tag="strip", name="qry_sb")
                    nc.sync.dma_start(qry_sb[:], qryT_r[:, :, qc * SQC:(qc + 1) * SQC])
                yield
                for cc in range(2):
                    pq = psp.tile([128, SQC], F32, tag="av", bufs=1, name="pq")
                    for d in range(DK):
                        nc.tensor.matmul(
                            pq[:],
                            wq_sb[:, d, cc * 128:(cc + 1) * 128],
                            qry_sb[:, d, :],
                            start=(d == 0), stop=(not with_bias and d == DK - 1),
                        )
                    if with_bias:
                        nc.tensor.matmul(
                            pq[:],
                            bq_sb[0:1, cc * 128:(cc + 1) * 128],
                            ones_sb[0:1, 0:SQC],
                            start=False, stop=True,
                        )
                    nc.vector.tensor_copy(
                        qt_all[:, cc, qc * SQC:(qc + 1) * SQC], pq[:]
                    )
                    yield
                qdone[qc] = True

            def gen_outproj(qc, otn, epilogue=False):
                odone[qc] = False
                for m in range(8):
                    ptag = ("av", "mm")[m % 2] if epilogue else "av"
                    pf = psp.tile([128, SQC], F32, tag=ptag, bufs=1 if ptag == "av" else 2, name="pf")
                    nc.tensor.matmul(
                        pf[:],
                        wout_sb[:, 0, m * 128:(m + 1) * 128],
                        otn[:, 0, :],
                        start=True, stop=False,
                    )
                    nc.tensor.matmul(
                        pf[:],
                        wout_sb[:, 1, m * 128:(m + 1) * 128],
                        otn[:, 1, :],
                        start=False, stop=True,
                    )
                    fin = workp.tile([128, SQC], BF16, tag="fin", bufs=4)
                    if epilogue:
                        nc.scalar.copy(fin[:], pf[:])
                    else:
                        nc.vector.tensor_copy(fin[:], pf[:])
                    nc.sync.dma_start(
                        outT_r[:, m, qc * SQC:(qc + 1) * SQC], fin[:]
                    )
                    yield
                odone[qc] = True

            filler = []
            qdone = {0: True}
            odone = {}

            def emit_filler(budget):
                while budget > 0 and filler:
                    try:
                        next(filler[0])
                        budget -= 1
                    except StopIteration:
                        filler.pop(0)

            emit_K_kT(0)
            for _ in gen_qproj(0, qry0_sb):
                pass
            kgen = gen_phaseK_rest()

            # 16 j-tiles per head in 6 groups; one activation per group
            # covers BOTH heads' scores (halves ACT instruction count).
            GROUPS = (3, 3, 3, 3, 2, 2)
            kdone = [1]  # K-jc0 emitted in the prologue
            otn_prev = None
            for qc in range(NSQC):
                if qc + 1 < NSQC:
                    qdone[qc + 1] = False
                    filler.append(gen_qproj(qc + 1))
                if otn_prev is not None:
                    filler.append(gen_outproj(qc - 1, otn_prev))
                # hard guarantees: qproj(qc) must be fully emitted before this
                # chunk's scores reference its qt columns, and outproj(qc-2)
                # before this chunk's otn slot (bufs=2 rotation) is rewritten
                # by the (front-inserted) norm fillers
                while not qdone[qc]:
                    emit_filler(50)
                while not odone.get(qc - 2, True):
                    emit_filler(50)
                qt = qt_all[:, :, qc * SQC:(qc + 1) * SQC]
                otn = workp.tile([128, 2, SQC], BF16, tag="otn", bufs=2)
                for pair in range(2):
                    startup = qc == 0 and pair == 0
                    he, ho = 2 * pair, 2 * pair + 1
                    # both heads' AV accumulators share one PSUM bank;
                    # allocated lazily at the first AV group so the bufs=1
                    # rotation waits on the previous pair's (already-emitted)
                    # ot copy
                    pav_h = [None]

                    def get_pav(pav_h=pav_h):
                        if pav_h[0] is None:
                            pav_h[0] = psp.tile(
                                [HD + 1, 2, SQC], F32, tag="pav", bufs=1, name="pav"
                            )
                        return pav_h[0]

                    def emit_av(prev, he=he, ho=ho):
                        gs0, jbase0, pt0_all = prev
                        pav = get_pav()
                        for hh, h, pt0 in ((0, he, pt0_all[:, 0]), (1, ho, pt0_all[:, 1])):
                            for sub in range(gs0):
                                j = jbase0 + sub
                                # start=True clears has_written for the WHOLE
                                # bank, so only the pair's first matmul may set
                                # it; head-odd's j=0 matmul overwrites its
                                # (bit-cleared) region via flags=0 semantics.
                                nc.tensor.matmul(
                                    pav[:, hh, :],
                                    v_sb[:, j, h, :],
                                    pt0[:, sub, :],
                                    start=(j == 0 and hh == 0),
                                    stop=(j == NJ - 1),
                                    skip_group_check=(j == 0 and hh == 1),
                                )

                    prev = None
                    jbase = 0
                    for gi, gs in enumerate(GROUPS):
                        if startup:
                            # emit K-jc sections before the groups needing them
                            need = (jbase + gs - 1) // 4
                            while kdone[0] <= need:
                                next(kgen)
                                kdone[0] += 1
                        ps = psp.tile([128, 2, 3, SQC], F32, tag="mm", bufs=2, name="ps")
                        # interleave the two heads' score matmuls so the two
                        # 64-row PE bands run concurrently
                        for sub in range(gs):
                            j = jbase + sub
                            nc.tensor.matmul(
                                ps[:, 0, sub, :],
                                kt_sb[0:64, pair, j * 128:(j + 1) * 128],
                                qt[0:64, pair, :],
                                start=True, stop=True,
                            )
                            nc.tensor.matmul(
                                ps[:, 1, sub, :],
                                kt_sb[64:128, pair, j * 128:(j + 1) * 128],
                                qt[64:128, pair, :],
                                start=True, stop=True,
                            )
                        pt = workp.tile([128, 2, 3, SQC], BF16, tag="pt", bufs=3, name="pt")
                        nc.scalar.activation(pt[:, :, 0:gs, :], ps[:, :, 0:gs, :], EXP)
                        if startup and gi == 0:
                            emit_K_v(0)
                        if prev is not None:
                            emit_av(prev)
                        prev = (gs, jbase, pt)
                        jbase += gs
                        if not startup:
                            emit_filler(2)
                    emit_av(prev)
                    if not startup:
                        emit_filler(1)

                    # normalization (divide by the keep-column accumulation)
                    # is deferred: emitted as front-of-queue filler during the
                    # next pair's groups, so the PE never waits on the DVE
                    # ot-copy / reciprocal chain
                    def gen_norm(pair=pair, pav=get_pav(), otn=otn):
                        ot = workp.tile([HD + 1, 2, SQC], F32, tag="ot", bufs=2)
                        nc.vector.tensor_copy(ot[:], pav[:])
                        rcp = workp.tile([1, 2, SQC], BF16, tag="rcp", bufs=2)
                        with nc.allow_low_precision(reason="bf16 reciprocal for softmax denom"):
                            nc.vector.reciprocal(rcp[:], ot[HD:HD + 1, :, :])
                        yield
                        # one matmul broadcasts both heads' reciprocals, and
                        # both muls (its readers) are emitted atomically with it
                        pbc = psp.tile([HD, 2, SQC], F32, tag="av", bufs=1, name="pbc")
                        nc.tensor.matmul(
                            pbc[:, :, :], ones_sb[0:1, 0:HD], rcp[0:1, :, :],
                            start=True, stop=True,
                        )
                        for hh in range(2):
                            po = hh * 64
                            nc.vector.tensor_mul(
                                otn[po:po + 64, pair, :], ot[0:HD, hh, :], pbc[:, hh, :]
                            )
                        yield

                    filler.insert(0, gen_norm())
                    if startup:
                        # keep the startup pair's norm from lagging behind the
                        # whole qproj(1) generator
                        emit_filler(2)
                otn_prev = otn

            # drain remaining filler, then the final chunk's out-projection
            emit_filler(10 ** 9)
            for _ in gen_outproj(NSQC - 1, otn_prev, epilogue=True):
                pass

    nc.compile()
    return nc


def _get_nc(with_bias=False):
    key = f"nc{int(with_bias)}"
    if key not in _CACHE:
        _CACHE[key] = _build(with_bias)
    return _CACHE[key]


LAST_RESULTS = None
LAST_IN_MAPS = None


def kernel(query, context, mask, Wq, bq, Wkv, bkv, Wout, bout, num_heads):
    import os
    import ml_dtypes
    from concourse.bass_utils import run_bass_kernel_spmd

    BF = ml_dtypes.bfloat16

    query = np.asarray(query, dtype=np.float32)
    context = np.asarray(context, dtype=np.float32)
    mask = np.asarray(mask)
    Wq = np.asarray(Wq, dtype=np.float32)
    bq_v = np.asarray(bq, dtype=np.float32)
    Wkv = np.asarray(Wkv, dtype=np.float32)
    bkv_v = np.asarray(bkv, dtype=np.float32)
    Wout = np.asarray(Wout, dtype=np.float32)
    bout_v = np.asarray(bout, dtype=np.float32)
    assert int(num_heads) == H

    scale = np.float32(HD ** -0.5)
    Wq_s = Wq * scale
    bq_s = bq_v * scale
    Wk = Wkv[:, :D]
    Wv = Wkv[:, D:]
    bk_v = bkv_v[:D]
    bv_v = bkv_v[D:]
    keep_f = 1.0 - mask.astype(np.float32)          # [B, SKV]
    ones_r = np.ones((1, SKC), dtype=BF)

    def bf(x):
        return np.ascontiguousarray(x.astype(BF))

    with_bias = bool(np.any(bq_s) or np.any(bk_v) or np.any(bv_v))
    nc = _get_nc(with_bias)
    in_maps = []
    for c in range(8):
        b, g = c // 4, c % 4
        cs = slice(g * COLS, (g + 1) * COLS)
        in_maps.append({
            "qryT": bf(query[b].T),
            "ctxT": bf(context[b].T),
            "wq": bf(Wq_s[:, cs]),
            "wk": bf(Wk[:, cs]),
            "wv": bf(Wv[:, cs]),
            "wout": bf(Wout[cs, :]),
            "bq": bf(bq_s[cs][None, :]),
            "bk": bf(bk_v[cs][None, :]),
            "bv": bf(bv_v[cs][None, :]),
            "ones": ones_r,
            "keep": np.ascontiguousarray(keep_f[b].reshape(NJ, 128).T),
        })

    trace = bool(int(os.environ.get("KERNEL_TRACE", "0")))
    res = run_bass_kernel_spmd(nc, in_maps, core_ids=list(range(8)), trace=trace)
    global LAST_RESULTS, LAST_IN_MAPS
    LAST_RESULTS = res
    LAST_IN_MAPS = in_maps

    out = np.empty((B, SQ, D), dtype=np.float32)
    for b in range(B):
        acc = np.zeros((D, SQ), dtype=np.float32)
        for g in range(4):
            acc += res.results[b * 4 + g]["outT"].astype(np.float32)
        out[b] = acc.T + bout_v[None, :]
    return out
